# revision 48
# baseline (speedup 1.0000x reference)
"""Llama decoder layer (prefill, GQA, SwiGLU) on 8 Trainium2 NeuronCores.

Tensor-parallel across 8 cores, chunk-pipelined so collectives and the
residual/norm stage overlap attention + MLP compute:
  - wq/wk/wv, w_gate/w_up column-sharded (4 q heads / 1 kv head / 1792 ffn
    per core); wo, w_down row-sharded.
  - o_proj partials carry x/8 so ReduceScatter(bf16) directly yields the
    residual shard (64 rows/core per 512-row chunk). Each core then does
    residual-write + RMSNorm2 on its shard only, transposes it, and an
    AllGather(bf16) redistributes the normalized chunk in [hid, seq] layout
    for the MLP.
  - MLP partials ReduceScatter(bf16) per chunk -> out_h shards.
  - ln1/ln2 folded into the following projection weights on host.
  - matmuls bf16 (fp32 PSUM accum); norms/softmax fp32.

kernel(**inputs) takes full unsharded fp32 inputs and returns
(h, residual) exactly like the reference decoder layer.
"""

import numpy as np
import ml_dtypes

import concourse.bass as bass
import concourse.mybir as mybir
import concourse.tile as tile
from concourse import bacc
from concourse.bass import ts, ds
from concourse.bass_utils import run_bass_kernel_spmd
from concourse.masks import make_identity

F32 = mybir.dt.float32
BF16 = mybir.dt.bfloat16
AF = mybir.ActivationFunctionType
ALU = mybir.AluOpType

HID = 4096
NH = 32
NKV = 8
HD = 128
G = 4            # q heads per kv head (= per core)
INTER = 14336
EPS = 1e-5
THETA = 10000.0
N_CORES = 8

FF = INTER // N_CORES       # 1792
FB = FF // 128              # 14 ffn 128-blocks
HC = HID // 128             # 32 hidden 128-chunks
EB = HID // 512             # 8 output 512-blocks
SCALING = float(HD) ** -0.5
CH = 512                    # rows per pipeline chunk
CHS = CH // N_CORES         # rows per core per chunk (64)
TPC = CH // 128             # s-tiles per chunk (4)


def _rmsnorm_stats(nc, pool_small, x_ap, P, eps=EPS):
    """Return rs_t [P,1] with x*rs_t = x/rms: rsqrt(sum(x^2)/HID + eps).

    For bf16 x = x_f32/8 inputs pass eps=EPS/64: then x8*rs_t equals the
    fp32-normalized row exactly. Squares on ScalarE -> SBUF scratch.
    """
    n_blk = x_ap.shape[-1] // 512
    zc = pool_small.tile([P, n_blk], F32, tag="rms_zc")
    for c in range(n_blk):
        sq = pool_small.tile([P, 512], F32, tag="rms_sq")
        nc.scalar.activation(sq[:], x_ap[:, ts(c, 512)], AF.Square,
                             accum_out=zc[:, c:c + 1])
    ssq = pool_small.tile([P, 1], F32, tag="rms_ssq")
    nc.vector.reduce_sum(ssq[:], zc[:], axis=mybir.AxisListType.X)
    rs_t = pool_small.tile([P, 1], F32, tag="rms_rs")
    nc.vector.tensor_scalar(rs_t[:], ssq[:], 1.0 / HID, eps, ALU.mult, ALU.add)
    nc.scalar.sqrt(rs_t[:], rs_t[:])
    nc.vector.reciprocal(rs_t[:], rs_t[:])
    return rs_t


def _build_program(S: int, no_collectives: bool = False):
    """Build the per-core Bass program (SPMD, rank-agnostic)."""
    T = S // 128
    NCH = S // CH                        # pipeline chunks (4)
    assert S % CH == 0

    nc = bacc.Bacc("TRN2", target_bir_lowering=False, debug=False,
                   num_devices=N_CORES)

    # ---- I/O ----
    hid_d = nc.dram_tensor("hidden", [S, HID], F32, kind="ExternalInput")
    hid8_d = nc.dram_tensor("hidden8", [S, HID], BF16, kind="ExternalInput")
    cos_d = nc.dram_tensor("cos_t", [S, 64], F32, kind="ExternalInput")
    sin_d = nc.dram_tensor("sin_t", [S, 64], F32, kind="ExternalInput")
    mask_d = nc.dram_tensor("mask_diag", [128, 128], F32, kind="ExternalInput")
    wqkv_d = nc.dram_tensor("wqkv_t", [128, HC, 768], BF16, kind="ExternalInput")
    wo_d = nc.dram_tensor("wo_t", [128, G, HID], BF16, kind="ExternalInput")
    wg_d = nc.dram_tensor("wg_t", [FB, 128, HC, 128], BF16, kind="ExternalInput")
    wu_d = nc.dram_tensor("wu_t", [FB, 128, HC, 128], BF16, kind="ExternalInput")
    wd_d = nc.dram_tensor("wd_t", [EB, 128, FB, 512], BF16, kind="ExternalInput")
    # per-core outputs: chunk-ordered shards (chunk j rows = global rows
    # j*CH + r*CHS + [0, CHS) for core r)
    out_h = nc.dram_tensor("out_h", [NCH * CHS, HID], BF16,
                           kind="ExternalOutput")
    out_res = nc.dram_tensor("out_res", [NCH * CHS, HID], BF16,
                             kind="ExternalOutput")

    # ---- internal DRAM (per-chunk, so collectives pipeline with compute) ----
    rs1_in = [nc.dram_tensor(f"rs1_in{j}", [CH, HID], BF16) for j in range(NCH)]
    rs1_out = [nc.dram_tensor(f"rs1_out{j}", [CHS, HID], BF16)
               for j in range(NCH)]
    ag_in = [nc.dram_tensor(f"ag_in{j}", [128, HC, CHS], BF16)
             for j in range(NCH)]
    ag_out = [nc.dram_tensor(f"ag_out{j}", [N_CORES, 128, HC, CHS], BF16,
                             addr_space="Shared") for j in range(NCH)]
    rs2_in = [nc.dram_tensor(f"rs2_in{j}", [CH, HID], BF16) for j in range(NCH)]
    rs2_out = [nc.dram_tensor(f"rs2_out{j}", [CHS, HID], BF16)
               for j in range(NCH)]
    rg = [list(range(N_CORES))]

    def coll(kind, op, in_t, out_t):
        if no_collectives:
            # sim-only stand-in; real collectives run on TOPSP rings, not
            # the kernel DMA queues, so keep these off the sync stream
            if kind == "AllGather":
                for r in range(N_CORES):
                    nc.scalar.dma_start(out_t[r], in_t[:])
            else:
                nc.scalar.dma_start(out_t[:], in_t[0:CHS, :])
        else:
            nc.gpsimd.collective_compute(kind, op, ins=[in_t[:]],
                                         outs=[out_t[:]], replica_groups=rg)

    def d_stage(dpool, j):
        """Residual shard + RMSNorm2 + transpose -> ag_in[j]; then AllGather."""
        rsb = dpool.tile([CHS, HID], BF16, tag="d_rsb")
        nc.sync.dma_start(rsb[:], rs1_out[j][:])
        nc.sync.dma_start(out_res[ts(j, CHS), :], rsb[:])
        rs_t = _rmsnorm_stats(nc, dpool, rsb[:], CHS)
        hn2 = dpool.tile([CHS, HID], BF16, tag="d_hn2")
        nc.vector.tensor_scalar_mul(hn2[:], rsb[:], rs_t[:])
        hn2T = dpool.tile([128, HC, CHS], BF16, tag="d_hn2T")
        nc.sync.dma_start_transpose(hn2T[:], hn2[:])
        nc.sync.dma_start(ag_in[j][:], hn2T[:])
        coll("AllGather", ALU.bypass, ag_in[j], ag_out[j])

    with tile.TileContext(nc) as tc:
        with (
            tc.tile_pool(name="persist", bufs=1) as persist,
        ):
            ident = persist.tile([128, 128], BF16)
            make_identity(nc, ident[:])
            mask_sb = persist.tile([128, 128], F32)
            nc.sync.dma_start(mask_sb[:], mask_d[:])

            # attention activations (live from stage AB through o_proj)
            with tc.tile_pool(name="attn_persist", bufs=1) as aper:
                qT_sb = aper.tile([128, G, S], BF16)
                kT_sb = aper.tile([128, S], BF16)
                v_sb = aper.tile([128, T, 128], BF16)
                attnT_sb = aper.tile([128, G, S], BF16)

                # ==== Stage AB: RMSNorm1 + QKV + RoPE, per s-tile ====
                with (
                    tc.tile_pool(name="stB", bufs=3) as stB,
                    tc.tile_pool(name="stBs", bufs=3) as stBs,
                    tc.tile_pool(name="stBw", bufs=1) as stBw,
                    tc.tile_pool(name="stBq", bufs=2, space="PSUM") as stBq,
                    tc.tile_pool(name="stBt", bufs=2, space="PSUM") as stBt,
                ):
                    wqkv_sb = stBw.tile([128, HC, 768], BF16, tag="wqkv")
                    for c8 in range(4):
                        nc.scalar.dma_start(wqkv_sb[:, ts(c8, 4), :],
                                            wqkv_d[:, ts(c8, 4), :])
                    for i in range(T):
                        xt = stB.tile([128, HID], BF16, tag="xt")
                        nc.sync.dma_start(xt[:, 0:2048],
                                          hid8_d[ts(i, 128), 0:2048])
                        nc.sync.dma_start(xt[:, 2048:HID],
                                          hid8_d[ts(i, 128), 2048:HID])
                        rs_t = _rmsnorm_stats(nc, stBs, xt[:], 128,
                                              eps=EPS / 64.0)
                        hn_bf = stB.tile([128, HID], BF16, tag="hnbf")
                        hnT_i = stB.tile([128, HC, 128], BF16, tag="hnT")
                        for qc in range(4):
                            nc.scalar.activation(hn_bf[:, ts(qc, 1024)],
                                                 xt[:, ts(qc, 1024)], AF.Copy,
                                                 scale=rs_t[:])
                            nc.sync.dma_start_transpose(
                                hnT_i[:, ts(qc, 8), :],
                                hn_bf[:, ts(qc, 1024)])
                        if i == 0:
                            # second wqkv half behind tile 0's transposes so
                            # its transfer doesn't delay the pipeline head
                            for c8 in range(4, 8):
                                nc.sync.dma_start(wqkv_sb[:, ts(c8, 4), :],
                                                  wqkv_d[:, ts(c8, 4), :])

                        pq = stBq.tile([128, 512], F32, tag="pq")
                        pkv = stBq.tile([128, 256], F32, tag="pkv")
                        for c in range(HC):
                            nc.tensor.matmul(pq[:], hnT_i[:, c, :],
                                             wqkv_sb[:, c, 0:512],
                                             start=(c == 0), stop=(c == HC - 1))
                        for c in range(HC):
                            nc.tensor.matmul(pkv[:], hnT_i[:, c, :],
                                             wqkv_sb[:, c, 512:768],
                                             start=(c == 0), stop=(c == HC - 1))
                        nc.vector.tensor_copy(v_sb[:, i, :], pkv[:, 128:256])

                        cs = stBs.tile([128, 64], F32, tag="cs")
                        sn = stBs.tile([128, 64], F32, tag="sn")
                        nc.sync.dma_start(cs[:], cos_d[ts(i, 128), :])
                        nc.sync.dma_start(sn[:], sin_d[ts(i, 128), :])

                        def rope(dst_bf, src_psum, nh):
                            s4 = src_psum.rearrange("p (h t d) -> p h t d",
                                                    h=nh, t=2)
                            d4 = dst_bf.rearrange("p (h t d) -> p h t d",
                                                  h=nh, t=2)
                            csb = cs[:, None, :].to_broadcast([128, nh, 64])
                            snb = sn[:, None, :].to_broadcast([128, nh, 64])
                            t1 = stBs.tile([128, nh, 64], F32, tag=f"rt1_{nh}")
                            t2 = stBs.tile([128, nh, 64], F32, tag=f"rt2_{nh}")
                            nc.vector.tensor_tensor(t1[:], s4[:, :, 0, :], csb,
                                                    ALU.mult)
                            nc.vector.tensor_tensor(t2[:], s4[:, :, 1, :], snb,
                                                    ALU.mult)
                            nc.vector.tensor_tensor(d4[:, :, 0, :], t1[:], t2[:],
                                                    ALU.subtract)
                            nc.vector.tensor_tensor(t1[:], s4[:, :, 1, :], csb,
                                                    ALU.mult)
                            nc.vector.tensor_tensor(t2[:], s4[:, :, 0, :], snb,
                                                    ALU.mult)
                            nc.vector.tensor_tensor(d4[:, :, 1, :], t1[:], t2[:],
                                                    ALU.add)

                        q_bf = stB.tile([128, 512], BF16, tag="qbf")
                        k_bf = stBs.tile([128, 128], BF16, tag="kbf")
                        rope(q_bf, pq, G)
                        rope(k_bf, pkv[:, 0:128], 1)
                        for h in range(G):
                            pt = stBt.tile([128, 128], BF16, tag="ptq")
                            nc.tensor.transpose(pt[:], q_bf[:, ts(h, 128)],
                                                ident[:])
                            nc.vector.tensor_copy(qT_sb[:, h, ts(i, 128)], pt[:])
                        pt = stBt.tile([128, 128], BF16, tag="ptq")
                        nc.tensor.transpose(pt[:], k_bf[:], ident[:])
                        nc.vector.tensor_copy(kT_sb[:, ts(i, 128)], pt[:])

                # ==== Stages C+O, chunk-pipelined with RS1 + D + AG ====
                with (
                    tc.tile_pool(name="stC", bufs=3) as stC,
                    tc.tile_pool(name="stCz", bufs=3) as stCz,
                    tc.tile_pool(name="stO", bufs=2) as stO,
                    tc.tile_pool(name="stOx", bufs=1) as stOx,
                    tc.tile_pool(name="stOw", bufs=1) as stOw,
                    tc.tile_pool(name="dsmall", bufs=1) as dsmall,
                    tc.tile_pool(name="stCs", bufs=2, space="PSUM") as psum_s,
                    tc.tile_pool(name="stCt", bufs=2, space="PSUM") as psum_t,
                    tc.tile_pool(name="stCa", bufs=2, space="PSUM") as psum_a,
                    tc.tile_pool(name="stOp", bufs=2, space="PSUM") as psum_o,
                ):
                    wo_sb = stOw.tile([128, G, HID], BF16, tag="wo")
                    for h in range(G):
                        nc.scalar.dma_start(wo_sb[:, h, :], wo_d[:, h, :])

                    for j in range(NCH):
                        # prefetch x/8 tiles for this chunk's o_proj
                        x8s = []
                        for i in range(j * TPC, (j + 1) * TPC):
                            x8 = stOx.tile([128, HID], BF16, tag=f"x8_{i % TPC}")
                            nc.sync.dma_start(x8[:], hid8_d[ts(i, 128), :])
                            x8s.append(x8)
                        # -- attention for tiles of chunk j --
                        for i in range(j * TPC, (j + 1) * TPC):
                            nk = i + 1
                            nb = (nk * 128 + 511) // 512
                            for h in range(G):
                                p_bf = stC.tile([128, S], BF16, tag="pbf")
                                zp = stCz.tile([128, 4], F32, tag="zp")
                                for b in range(nb):
                                    klo = b * 512
                                    khi = min(nk * 128, klo + 512)
                                    w = khi - klo
                                    sc = psum_s.tile([128, 512], F32, tag="sc")
                                    nc.tensor.matmul(sc[:, 0:w],
                                                     qT_sb[:, h, ts(i, 128)],
                                                     kT_sb[:, klo:khi],
                                                     start=True, stop=True)
                                    if klo <= i * 128 < khi:
                                        off = i * 128 - klo
                                        nc.vector.tensor_tensor(
                                            sc[:, off:off + 128],
                                            sc[:, off:off + 128],
                                            mask_sb[:], ALU.add)
                                    nc.scalar.activation(
                                        p_bf[:, klo:khi], sc[:, 0:w],
                                        AF.Exp, scale=SCALING,
                                        accum_out=zp[:, b:b + 1])
                                z = stCz.tile([128, 1], F32, tag="z")
                                nc.vector.reduce_sum(z[:], zp[:, 0:nb],
                                                     axis=mybir.AxisListType.X)
                                nc.vector.reciprocal(z[:], z[:])
                                pa = psum_a.tile([128, 128], F32, tag="pa")
                                for g in range((nk + 3) // 4):
                                    k0 = g * 4
                                    kw = min(4, nk - k0)
                                    ptp = psum_t.tile([128, 4, 128], BF16,
                                                      tag="ptp")
                                    for u in range(kw):
                                        nc.tensor.transpose(
                                            ptp[:, u, :],
                                            p_bf[:, ts(k0 + u, 128)], ident[:])
                                    pT = stC.tile([128, 4, 128], BF16,
                                                  tag="pT")
                                    nc.vector.tensor_copy(pT[:, 0:kw, :],
                                                          ptp[:, 0:kw, :])
                                    for u in range(kw):
                                        nc.tensor.matmul(pa[:], pT[:, u, :],
                                                         v_sb[:, k0 + u, :],
                                                         start=(k0 + u == 0),
                                                         stop=(k0 + u == nk - 1))
                                a_bf = stC.tile([128, 128], BF16, tag="abf")
                                nc.vector.tensor_scalar_mul(a_bf[:], pa[:], z[:])
                                pt2 = psum_t.tile([128, 128], BF16, tag="ptp")
                                nc.tensor.transpose(pt2[:], a_bf[:], ident[:])
                                nc.vector.tensor_copy(
                                    attnT_sb[:, h, ts(i, 128)], pt2[:])

                        # -- o_proj for chunk j (partials + x/8 -> rs1_in) --
                        for i in range(j * TPC, (j + 1) * TPC):
                            x8 = x8s[i % TPC]
                            ot = stO.tile([128, HID], BF16, tag="ot")
                            for e in range(EB):
                                po = psum_o.tile([128, 512], F32, tag="po")
                                for h in range(G):
                                    nc.tensor.matmul(po[:],
                                                     attnT_sb[:, h, ts(i, 128)],
                                                     wo_sb[:, h, ts(e, 512)],
                                                     start=(h == 0),
                                                     stop=(h == G - 1))
                                nc.vector.tensor_tensor(ot[:, ts(e, 512)],
                                                        po[:], x8[:, ts(e, 512)],
                                                        ALU.add)
                            nc.sync.dma_start(
                                rs1_in[j][ts(i % TPC, 128), 0:2048],
                                ot[:, 0:2048])
                            nc.sync.dma_start(
                                rs1_in[j][ts(i % TPC, 128), 2048:HID],
                                ot[:, 2048:HID])
                        coll("ReduceScatter", ALU.add, rs1_in[j], rs1_out[j])

                        # -- D stage for chunks 0,1 (2,3 run in the E region
                        # so this pool's close barrier never gates E) --
                        if j in (1, 2):
                            d_stage(dsmall, j - 1)

            # ==== Stage E: MLP per chunk; D(last) hidden under E(0) ====
            with (
                tc.tile_pool(name="stEh", bufs=2) as stEh,
                tc.tile_pool(name="stEw", bufs=2) as stEw,
                tc.tile_pool(name="stEwd", bufs=2) as stEwd,
                tc.tile_pool(name="stEg", bufs=2) as stEg,
                tc.tile_pool(name="stE", bufs=3) as stE,
                tc.tile_pool(name="dsmall2", bufs=1) as dsmall2,
                tc.tile_pool(name="stEp", bufs=2, space="PSUM") as psum_g,
                tc.tile_pool(name="stEd", bufs=2, space="PSUM") as psum_d,
            ):
                def fetch_h2(j):
                    # h2 SBUF layout [p, r, c, t]: rank-major so both DMA APs
                    # balance to 3 dims; seq order (r,t) is natural.
                    h2_4d = stEh.tile([128, N_CORES, HC, CHS], BF16, tag="h2")
                    src = ag_out[j][:].rearrange("r p c t -> p r c t")
                    dst = h2_4d[:]
                    for r in range(N_CORES):
                        nc.scalar.dma_start(dst[:, r, :, :],
                                            src[:, r, :, :])
                    return h2_4d

                wgwu_pre = []
                for f in range(2):
                    wg0 = stEw.tile([128, HC, 128], BF16, tag="wg")
                    wu0 = stEw.tile([128, HC, 128], BF16, tag="wu")
                    nc.scalar.dma_start(wg0[:], wg_d[f])
                    nc.scalar.dma_start(wu0[:], wu_d[f])
                    wgwu_pre.append((wg0, wu0))
                h2_next = fetch_h2(0)
                for j in range(NCH):
                    h2v = h2_next[:]
                    guT = stEg.tile([128, FB, CH], BF16, tag="guT")
                    for f in range(FB):
                        if j == 0 and f < 2:
                            wg_sb, wu_sb = wgwu_pre[f]
                        else:
                            wg_sb = stEw.tile([128, HC, 128], BF16, tag="wg")
                            wu_sb = stEw.tile([128, HC, 128], BF16, tag="wu")
                            nc.scalar.dma_start(wg_sb[:], wg_d[f])
                            nc.scalar.dma_start(wu_sb[:], wu_d[f])
                        pg = psum_g.tile([128, 512], F32, tag="pg")
                        pu = psum_g.tile([128, 512], F32, tag="pu")
                        for c in range(HC):
                            nc.tensor.matmul(pg[:], wg_sb[:, c, :],
                                             h2v[:, :, c, :],
                                             start=(c == 0), stop=(c == HC - 1))
                        for c in range(HC):
                            nc.tensor.matmul(pu[:], wu_sb[:, c, :],
                                             h2v[:, :, c, :],
                                             start=(c == 0), stop=(c == HC - 1))
                        sil = stE.tile([128, 512], F32, tag="sil")
                        nc.scalar.activation(sil[:], pg[:], AF.Silu)
                        nc.vector.tensor_tensor(guT[:, f, :], sil[:], pu[:],
                                                ALU.mult)
                    if j in (0, 1):
                        # D stages for chunks 2,3 hidden under E(0)/E(1)
                        d_stage(dsmall2, j + 2)
                    if j + 1 < NCH:
                        h2_next = fetch_h2(j + 1)
                    for e in range(EB):
                        wd_sb = stEwd.tile([128, FB, 512], BF16, tag="wd")
                        nc.sync.dma_start(wd_sb[:], wd_d[e])
                        for ti in range(TPC):
                            pd = psum_d.tile([128, 512], F32, tag="pd")
                            for f in range(FB):
                                nc.tensor.matmul(pd[:], guT[:, f, ts(ti, 128)],
                                                 wd_sb[:, f, :],
                                                 start=(f == 0),
                                                 stop=(f == FB - 1))
                            od = stE.tile([128, 512], BF16, tag="od")
                            nc.vector.tensor_copy(od[:], pd[:])
                            nc.sync.dma_start(
                                rs2_in[j][ts(ti, 128), ts(e, 512)], od[:])
                    coll("ReduceScatter", ALU.add, rs2_in[j], rs2_out[j])
                    nc.sync.dma_start(out_h[ts(j, CHS), :], rs2_out[j][:])

    nc.compile()
    return nc


_PROGRAM_CACHE = {}


def _get_program(S, no_collectives=False):
    key = (S, no_collectives)
    if key not in _PROGRAM_CACHE:
        _PROGRAM_CACHE[key] = _build_program(S, no_collectives)
    return _PROGRAM_CACHE[key]


def _prep_inputs(positions, hidden_states, wq, wk, wv, wo,
                 w_gate, w_up, w_down, ln1_w, ln2_w):
    """Shard + retile + cast weights per core. Returns list of in_maps."""
    bf = ml_dtypes.bfloat16
    pos = np.asarray(positions, np.float32)
    half = HD // 2
    inv_freq = 1.0 / (THETA ** (np.arange(half, dtype=np.float32) * 2.0 / HD))
    freqs = pos[:, None] * inv_freq[None, :]
    cos_t = np.cos(freqs).astype(np.float32)
    sin_t = np.sin(freqs).astype(np.float32)
    qi = np.arange(128)
    mask_diag = np.where(qi[:, None] >= qi[None, :], 0.0, -1e9).astype(np.float32)

    ln1 = np.asarray(ln1_w, np.float32)[:, None]
    ln2 = np.asarray(ln2_w, np.float32)[:, None]
    wq_f = (np.asarray(wq) * ln1).astype(bf)
    wk_f = (np.asarray(wk) * ln1).astype(bf)
    wv_f = (np.asarray(wv) * ln1).astype(bf)
    wg_f = (np.asarray(w_gate) * ln2).astype(bf)
    wu_f = (np.asarray(w_up) * ln2).astype(bf)
    wo_f = np.asarray(wo).astype(bf)
    wd_f = np.asarray(w_down).astype(bf)
    hid = np.ascontiguousarray(np.asarray(hidden_states, np.float32))
    hid8 = (hid * 0.125).astype(bf)

    maps = []
    for r in range(N_CORES):
        wq_r = wq_f[:, r * 512:(r + 1) * 512]
        wk_r = wk_f[:, r * 128:(r + 1) * 128]
        wv_r = wv_f[:, r * 128:(r + 1) * 128]
        wqkv = np.concatenate([wq_r, wk_r, wv_r], axis=1)        # [4096, 768]
        wqkv_t = np.ascontiguousarray(
            wqkv.reshape(HC, 128, 768).transpose(1, 0, 2))       # [128, 32, 768]
        wo_r = wo_f[r * 512:(r + 1) * 512, :]                    # [512, 4096]
        wo_t = np.ascontiguousarray(
            wo_r.reshape(G, 128, HID).transpose(1, 0, 2))        # [128, 4, 4096]
        wg_r = wg_f[:, r * FF:(r + 1) * FF]                      # [4096, 1792]
        wu_r = wu_f[:, r * FF:(r + 1) * FF]
        wg_t = np.ascontiguousarray(
            wg_r.reshape(HC, 128, FB, 128).transpose(2, 1, 0, 3))
        wu_t = np.ascontiguousarray(
            wu_r.reshape(HC, 128, FB, 128).transpose(2, 1, 0, 3))
        wd_r = wd_f[r * FF:(r + 1) * FF, :]                      # [1792, 4096]
        wd_t = np.ascontiguousarray(
            wd_r.reshape(FB, 128, EB, 512).transpose(2, 1, 0, 3))
        maps.append({
            "hidden": hid, "hidden8": hid8, "cos_t": cos_t, "sin_t": sin_t,
            "mask_diag": mask_diag, "wqkv_t": wqkv_t, "wo_t": wo_t,
            "wg_t": wg_t, "wu_t": wu_t, "wd_t": wd_t,
        })
    return maps


def kernel(positions, hidden_states, wq, wk, wv, wo,
           w_gate, w_up, w_down, ln1_w, ln2_w):
    S = np.asarray(hidden_states).shape[0]
    nc = _get_program(S)
    maps = _prep_inputs(positions, hidden_states, wq, wk, wv, wo,
                        w_gate, w_up, w_down, ln1_w, ln2_w)
    res = run_bass_kernel_spmd(nc, maps, list(range(N_CORES)))
    NCH = S // CH
    h = np.empty((S, HID), np.float32)
    residual = np.empty((S, HID), np.float32)
    for r in range(N_CORES):
        hr = np.asarray(res.results[r]["out_h"]).astype(np.float32)
        rr = np.asarray(res.results[r]["out_res"]).astype(np.float32)
        for j in range(NCH):
            rows = slice(j * CH + r * CHS, j * CH + (r + 1) * CHS)
            residual[rows] = rr[j * CHS:(j + 1) * CHS]
            h[rows] = hr[j * CHS:(j + 1) * CHS]
    return h, residual
